# revision 31
# baseline (speedup 1.0000x reference)
"""Trainium2 Bass kernel for nn_ExtremeOptimizationLayer (64-branch MLP + per-branch
BatchNorm + fusion gate), SPMD across 8 NeuronCores.

Sharding: expert-parallel over the 64 branches (8 per core). Per core:
  GEMM1: h_k = relu(x @ W_k + b_k) for the 8 local branches, full batch,
         computing BN batch stats on the fly (bn_stats/bn_aggr).
  BN is folded into the fusion GEMM: h_norm = h*u + v with
         u = gamma*rsqrt(var+eps), v = beta - mean*u, so
         z_partial = h @ (u*Wf1_local) + (v @ Wf1_local).
  The rank-1 term zc = v @ Wf1_local is computed with column-packed M=1
  matmuls (4 concurrent PE column groups), summed across cores by a tiny
  AllReduce, and folded into the post-RS relu bias.
  GEMM2: z_partial[b, j] over the local 8192 rows of Wf1.
  ReduceScatter (16 x 256-row pieces, overlapped with compute) sums the 8
  partial z's; each core ends with 256 batch rows, applies relu(z+bf1+zc)
  and the final GEMM with Wf2.
All matmuls run in bf16 (fp32 PSUM accumulation).

DMA queue split (avoids head-of-line blocking): weights/consts on gpsimd,
xT + h traffic on sync, z/zs/out traffic on scalar.
"""

import numpy as np
import ml_dtypes

import concourse.bass as bass
import concourse.mybir as mybir
import concourse.tile as tile
from concourse import bacc
from concourse.bass_utils import run_bass_kernel_spmd
from concourse.masks import make_identity

F32 = mybir.dt.float32
BF16 = mybir.dt.bfloat16
BD = ml_dtypes.bfloat16
AF = mybir.ActivationFunctionType

FULL_CFG = dict(
    n_cores=8, B=2048, DI=1024, DO=1024, KT=64, DF=1024, DO2=1024,
    JCH=2, EPS=1e-5,
)


def _dims(cfg):
    d = dict(cfg)
    d["KB"] = cfg["KT"] // cfg["n_cores"]        # branches per core
    d["TPB"] = cfg["DO"] // 128                  # o-tiles per branch
    d["T"] = d["KB"] * d["TPB"]                  # local ko tiles
    d["NIT"] = cfg["DI"] // 128                  # i-tiles (GEMM1 contraction)
    d["BC"] = min(512, cfg["B"])                 # GEMM1 batch chunk
    d["NBC"] = cfg["B"] // d["BC"]
    d["JW"] = cfg["DF"] // cfg["JCH"]            # j-chunk width
    d["BMW"] = min(512, cfg["B"])                # GEMM2 b-macro width
    d["NBM"] = cfg["B"] // d["BMW"]
    d["NBT"] = d["BMW"] // 128                   # b-tiles per macro
    d["PR"] = min(256, d["BMW"])                 # RS piece rows
    d["NPS"] = d["BMW"] // d["PR"]               # RS pieces per b-macro
    d["BL"] = cfg["B"] // cfg["n_cores"]         # local batch rows after RS
    d["PL"] = d["PR"] // cfg["n_cores"]          # local rows per RS piece
    d["NJT"] = cfg["DF"] // 128                  # final j-tiles
    d["NCT"] = cfg["DO2"] // 128                 # final out-col tiles
    d["NBLT"] = (d["BL"] + 127) // 128           # final local-b tiles
    d["HQ"] = min(4, d["T"])                     # ko-tiles per h2 macro-DMA
    return d


def build_bass(cfg):
    d = _dims(cfg)
    n_cores, B, DI, DO = cfg["n_cores"], cfg["B"], cfg["DI"], cfg["DO"]
    DF, DO2, JCH, EPS = cfg["DF"], cfg["DO2"], cfg["JCH"], cfg["EPS"]
    KB, TPB, T, NIT = d["KB"], d["TPB"], d["T"], d["NIT"]
    BC, NBC, JW = d["BC"], d["NBC"], d["JW"]
    BMW, NBM, NBT, BL, PL = d["BMW"], d["NBM"], d["NBT"], d["BL"], d["PL"]
    PR, NPS, HQ = d["PR"], d["NPS"], d["HQ"]
    NJT, NCT, NBLT = d["NJT"], d["NCT"], d["NBLT"]

    nc = bacc.Bacc("TRN2", target_bir_lowering=False, debug=False,
                   num_devices=n_cores)

    xT = nc.dram_tensor("xT", [DI, B], BF16, kind="ExternalInput").ap()
    wloc = nc.dram_tensor("wloc", [KB * DI, DO], BF16, kind="ExternalInput").ap()
    w1loc = nc.dram_tensor("w1loc", [KB * DO, DF], BF16, kind="ExternalInput").ap()
    wf2 = nc.dram_tensor("wf2", [DF, DO2], BF16, kind="ExternalInput").ap()
    b_r = nc.dram_tensor("b_r", [128, T], F32, kind="ExternalInput").ap()
    gamma_r = nc.dram_tensor("gamma_r", [128, T], F32, kind="ExternalInput").ap()
    beta_r = nc.dram_tensor("beta_r", [128, T], F32, kind="ExternalInput").ap()
    bf1_r = nc.dram_tensor("bf1_r", [128, NJT], F32, kind="ExternalInput").ap()
    bf2_r = nc.dram_tensor("bf2_r", [128, NCT], F32, kind="ExternalInput").ap()
    outT = nc.dram_tensor("outT", [DO2, BL], F32, kind="ExternalOutput").ap()

    h_d = nc.dram_tensor("h_d", [KB * DO, B], BF16, kind="Internal").ap()
    # RS pieces: (j-chunk, b-macro, sub-piece) -> PR rows of z
    zp = {}
    zs = {}
    for jc in range(JCH):
        for bm in range(NBM):
            for sp in range(NPS):
                zp[(jc, bm, sp)] = nc.dram_tensor(
                    f"zp{jc}_{bm}_{sp}", [PR, JW], F32, kind="Internal").ap()
                zs[(jc, bm, sp)] = nc.dram_tensor(
                    f"zs{jc}_{bm}_{sp}", [PL, JW], F32, kind="Internal").ap()
    zc_in = nc.dram_tensor("zc_in", [4, DF], F32, kind="Internal").ap()
    zc_out = nc.dram_tensor("zc_out", [4, DF], F32, kind="Internal").ap()

    with tile.TileContext(nc) as tc:
        with tc.tile_pool(name="const", bufs=1) as cp, \
             tc.tile_pool(name="stats", bufs=1) as sp_pool, \
             tc.tile_pool(name="w1a", bufs=T) as w1a, \
             tc.tile_pool(name="h2", bufs=4) as h2p:
            br_sb = cp.tile([128, T], F32, name="br_sb")
            gr_sb = cp.tile([128, T], F32, name="gr_sb")
            be_sb = cp.tile([128, T], F32, name="be_sb")
            bf1_sb = cp.tile([128, NJT], F32, name="bf1_sb")
            bf2_sb = cp.tile([128, NCT], F32, name="bf2_sb")
            eps_sb = cp.tile([128, 1], F32, name="eps_sb")
            ident = cp.tile([128, 128], F32, name="ident")
            wf2_sb = cp.tile([128, NJT, DO2], BF16, name="wf2_sb")

            # stats
            mv = sp_pool.tile([128, T, 2], F32, name="mv")
            u_all = sp_pool.tile([128, T], F32, name="u_all")
            v_f32 = sp_pool.tile([128, T], F32, name="v_f32")
            v_bf = sp_pool.tile([128, T], BF16, name="v_bf")
            zcs_t = [sp_pool.tile([128, JW], F32, name=f"zcs_{jc}")
                     for jc in range(JCH)]

            w1_tiles = [[None] * T for _ in range(JCH)]

            def zc_pack_mm(zc_ps, jc, t):
                """zc[j] += v[t-tile] @ Wf1[t-tile, jc-chunk] as an M=1 matmul
                packed into PE column group t%4 (concurrent execution)."""
                g = t % 4
                nc.tensor.matmul(zc_ps[32 * g:32 * g + 1, :], v_bf[:, t:t + 1],
                                 w1_tiles[jc][t][:],
                                 start=(t < 4), stop=(t >= T - 4),
                                 tile_position=(0, 32 * g),
                                 skip_group_check=True)

            def zc_collect(zc_ps, zcs, jc):
                """Copy the zc PSUM bank to SBUF (lane-aligned), then DMA the 4
                packed rows (partitions 0/32/64/96) to the AllReduce input."""
                nc.scalar.activation(zcs[:], zc_ps[:], AF.Copy)
                rows = zcs[:].rearrange("(g s) w -> g s w", s=32)[:, 0:1, :]
                nc.scalar.dma_start(zc_in[0:4, jc * JW:(jc + 1) * JW], rows)

            # ---------------- GEMM1: branch MLPs + BN stats ----------------
            with tc.tile_pool(name="xt", bufs=1) as xtp, \
                 tc.tile_pool(name="w", bufs=2 * NIT) as wp, \
                 tc.tile_pool(name="h1", bufs=8) as hp, \
                 tc.tile_pool(name="bn", bufs=2 * TPB + 2) as bnp, \
                 tc.tile_pool(name="g1ps", bufs=6, space="PSUM") as g1ps, \
                 tc.tile_pool(name="zc0ps", bufs=1, space="PSUM") as zc0ps:
                # branch-0 weights split across two queues for fast start
                w_tiles = []
                for it in range(NIT):
                    wt = wp.tile([128, DO], BF16, name=f"w_0_{it}", tag="w")
                    eng = nc.sync if it == 0 else (
                        nc.gpsimd if it % 2 == 0 else nc.scalar)
                    eng.dma_start(wt[:], wloc[it * 128:(it + 1) * 128, :])
                    w_tiles.append(wt)
                nc.gpsimd.dma_start(br_sb[:], b_r[:, :])
                nc.gpsimd.dma_start(gr_sb[:], gamma_r[:, :])
                nc.gpsimd.dma_start(be_sb[:], beta_r[:, :])
                nc.gpsimd.memset(eps_sb[:], EPS)
                # xT on the sync queue, bc-major so the first MMs unblock fast
                xt_sb = xtp.tile([128, NIT, B], BF16, name="xt_sb")
                for bc in range(NBC):
                    for it in range(NIT):
                        nc.sync.dma_start(
                            xt_sb[:, it, bc * BC:(bc + 1) * BC],
                            xT[it * 128:(it + 1) * 128, bc * BC:(bc + 1) * BC])
                zc0_ps = zc0ps.tile([128, JW], F32, name="zc0_ps")
                nc.vector.memset(zc0_ps[:], 0.0)

                for kb in range(KB):
                    if kb > 0:
                        w_tiles = []
                        for it in range(NIT):
                            wt = wp.tile([128, DO], BF16, name=f"w_{kb}_{it}",
                                         tag="w")
                            nc.gpsimd.dma_start(
                                wt[:],
                                wloc[kb * DI + it * 128:kb * DI + (it + 1) * 128, :])
                            w_tiles.append(wt)
                    # spread Wf1 chunk-0 prefetch across branches (gpsimd)
                    for ot in range(TPB):
                        t = kb * TPB + ot
                        w1t = w1a.tile([128, JW], BF16, name=f"w1_0_{t}",
                                       tag="w1a")
                        nc.gpsimd.dma_start(
                            w1t[:], w1loc[t * 128:(t + 1) * 128, 0:JW])
                        w1_tiles[0][t] = w1t
                    bn6s = [bnp.tile([128, NBC, 6], F32,
                                     name=f"bn6_{kb * TPB + ot}", tag="bn6")
                            for ot in range(TPB)]
                    # branch 0 runs bc-outer so the first matmuls only need the
                    # first xT batch-chunk; later branches run ot-outer
                    if kb == 0:
                        loop = [(ot, bc) for bc in range(NBC) for ot in range(TPB)]
                    else:
                        loop = [(ot, bc) for ot in range(TPB) for bc in range(NBC)]
                    for ot, bc in loop:
                        t = kb * TPB + ot
                        ps = g1ps.tile([128, BC], F32, name=f"g1_{t}_{bc}",
                                       tag="g1")
                        for it in range(NIT):
                            nc.tensor.matmul(
                                ps[:],
                                w_tiles[it][:, ot * 128:(ot + 1) * 128],
                                xt_sb[:, it, bc * BC:(bc + 1) * BC],
                                start=(it == 0), stop=(it == NIT - 1))
                        hsb = hp.tile([128, BC], BF16, name=f"h_{t}_{bc}",
                                      tag="h1")
                        nc.scalar.activation(hsb[:], ps[:], AF.Relu,
                                             bias=br_sb[:, t:t + 1])
                        nc.vector.bn_stats(bn6s[ot][:, bc, :], hsb[:])
                        nc.scalar.dma_start(
                            h_d[t * 128:(t + 1) * 128, bc * BC:(bc + 1) * BC],
                            hsb[:])
                    for ot in range(TPB):
                        t = kb * TPB + ot
                        nc.vector.bn_aggr(
                            mv[:, t, :],
                            bn6s[ot][:].rearrange("p a (x c) -> p (a x) c", c=3))
                    # per-branch BN affine folding: u = gamma*rsqrt(var+eps),
                    # v = beta - mean*u
                    t0 = kb * TPB
                    stdt = bnp.tile([128, TPB], F32, name=f"std_{kb}", tag="std")
                    nc.scalar.activation(stdt[:], mv[:, t0:t0 + TPB, 1:2],
                                         AF.Sqrt, bias=eps_sb[:])
                    invt = bnp.tile([128, TPB], F32, name=f"inv_{kb}", tag="inv")
                    nc.vector.reciprocal(invt[:], stdt[:])
                    nc.vector.tensor_mul(u_all[:, t0:t0 + TPB], invt[:],
                                         gr_sb[:, t0:t0 + TPB])
                    mut = bnp.tile([128, TPB], F32, name=f"mu_{kb}", tag="mu")
                    nc.vector.tensor_mul(mut[:], mv[:, t0:t0 + TPB, 0:1],
                                         u_all[:, t0:t0 + TPB])
                    nc.vector.tensor_sub(v_f32[:, t0:t0 + TPB],
                                         be_sb[:, t0:t0 + TPB], mut[:])
                    nc.vector.tensor_copy(v_bf[:, t0:t0 + TPB],
                                          v_f32[:, t0:t0 + TPB])
                    # chunk-0 prep (zc, then fold u into Wf1) for the PREVIOUS
                    # branch — its stats chain has finished by now, so the
                    # in-order PE doesn't stall on it.
                    for pb in ([kb - 1] if kb > 0 else []) + \
                              ([kb] if kb == KB - 1 else []):
                        for ot in range(TPB):
                            t = pb * TPB + ot
                            zc_pack_mm(zc0_ps, 0, t)
                            nc.vector.tensor_scalar_mul(w1_tiles[0][t][:],
                                                        w1_tiles[0][t][:],
                                                        u_all[:, t:t + 1])
                zc_collect(zc0_ps, zcs_t[0], 0)

            # ---------------- GEMM2: fusion gate partials + RS ----------------
            with tc.tile_pool(name="w1b", bufs=(JCH - 1) * T if JCH > 1 else 1) as w1b, \
                 tc.tile_pool(name="zsb", bufs=3) as zsbp, \
                 tc.tile_pool(name="zr", bufs=1) as zrp, \
                 tc.tile_pool(name="fo", bufs=3) as fop, \
                 tc.tile_pool(name="zps", bufs=6, space="PSUM") as zps, \
                 tc.tile_pool(name="fin_ps", bufs=2, space="PSUM") as finp:
                # consts needed from the middle of GEMM2 on (gpsimd queue)
                nc.gpsimd.dma_start(bf1_sb[:], bf1_r[:, :])
                nc.gpsimd.dma_start(bf2_sb[:], bf2_r[:, :])
                make_identity(nc, ident[:])
                # later chunks' Wf1 DMA (gpsimd queue, ahead of any RS)
                for jc in range(1, JCH):
                    for t in range(T):
                        w1t = w1b.tile([128, JW], BF16, name=f"w1_{jc}_{t}",
                                       tag="w1b")
                        nc.gpsimd.dma_start(
                            w1t[:],
                            w1loc[t * 128:(t + 1) * 128, jc * JW:(jc + 1) * JW])
                        w1_tiles[jc][t] = w1t
                for jt in range(NJT):
                    nc.gpsimd.dma_start(wf2_sb[:, jt, :],
                                        wf2[jt * 128:(jt + 1) * 128, :])

                # final-phase state (filled in as RS pieces land)
                zs_sb = []
                for blt in range(NBLT):
                    blw = min(128, BL - blt * 128)
                    zt = zrp.tile([128, DF], F32, name=f"zs_sb_{blt}",
                                  tag=f"zs_sb{blt}")
                    zs_sb.append((zt, blw))
                zcb4 = zrp.tile([128, 4, NJT], F32, name="zcb4")
                biasall = zrp.tile([128, NJT], F32, name="biasall")
                zrT = [zrp.tile([128, BL], BF16, name=f"zrT_{jt}",
                                tag=f"zrT{jt}") for jt in range(NJT)]
                transposed = set()

                def transpose_relu_blk(jt, blt):
                    zt, blw = zs_sb[blt]
                    tp = finp.tile([128, 128], F32, name=f"tp_{jt}_{blt}",
                                   tag="fin")
                    nc.tensor.transpose(tp[:, 0:blw],
                                        zt[0:blw, jt * 128:(jt + 1) * 128],
                                        ident[0:blw, 0:blw])
                    nc.scalar.activation(zrT[jt][:, blt * 128:blt * 128 + blw],
                                         tp[:, 0:blw], AF.Relu,
                                         bias=biasall[:, jt:jt + 1])

                def transpose_relu(jt):
                    for blt in range(len(zs_sb)):
                        transpose_relu_blk(jt, blt)

                for jc in range(JCH):
                    for bm in range(NBM):
                        z_ps = [zps.tile([128, JW], F32, name=f"z_{jc}_{bm}_{bt}",
                                         tag="z")
                                for bt in range(NBT)]
                        for tq in range(T // HQ):
                            ht = h2p.tile([128, HQ, BMW], BF16,
                                          name=f"h2_{jc}_{bm}_{tq}", tag="h2")
                            nc.sync.dma_start(
                                ht[:],
                                h_d[tq * HQ * 128:(tq + 1) * HQ * 128,
                                    bm * BMW:(bm + 1) * BMW]
                                .rearrange("(q p) b -> p q b", p=128))
                            for q in range(HQ):
                                t = tq * HQ + q
                                for bt in range(NBT):
                                    nc.tensor.matmul(
                                        z_ps[bt][:],
                                        ht[:, q, bt * 128:(bt + 1) * 128],
                                        w1_tiles[jc][t][:],
                                        start=(t == 0), stop=(t == T - 1),
                                        skip_group_check=True)
                        for bt in range(NBT):
                            zsb = zsbp.tile([128, JW], F32,
                                            name=f"zsb_{jc}_{bm}_{bt}", tag="zsb")
                            nc.vector.tensor_copy(zsb[:], z_ps[bt][:])
                            sp_i = (bt * 128) // PR
                            ro = (bt * 128) % PR
                            nc.scalar.dma_start(
                                zp[(jc, bm, sp_i)][ro:ro + 128, :], zsb[:])
                            if ro + 128 == PR:
                                nc.gpsimd.collective_compute(
                                    "ReduceScatter", mybir.AluOpType.add,
                                    replica_groups=[list(range(n_cores))],
                                    ins=[zp[(jc, bm, sp_i)].opt()],
                                    outs=[zs[(jc, bm, sp_i)].opt()])
                                # pull the summed piece into the final-z tile
                                l0 = (bm * NPS + sp_i) * PL
                                blt, ro2 = l0 // 128, l0 % 128
                                nc.gpsimd.dma_start(
                                    zs_sb[blt][0][ro2:ro2 + PL,
                                                  jc * JW:(jc + 1) * JW],
                                    zs[(jc, bm, sp_i)][:, :])
                        # prep of next chunk's zc + scale folding, right
                        # after this chunk's first b-macro (tiles have arrived,
                        # and the folded tiles are ready well before chunk nj)
                        nj = jc + 1
                        if bm == 0 and nj < JCH:
                            zcn_ps = finp.tile([128, JW], F32, name=f"zc_{nj}",
                                               tag="fin")
                            nc.vector.memset(zcn_ps[:], 0.0)
                            for t in range(T):
                                zc_pack_mm(zcn_ps, nj, t)
                            for t in range(T):
                                nc.vector.tensor_scalar_mul(
                                    w1_tiles[nj][t][:], w1_tiles[nj][t][:],
                                    u_all[:, t:t + 1])
                            zc_collect(zcn_ps, zcs_t[nj], nj)
                        # interleave the first j-half's transposes midway
                        # through the second chunk (their RS long finished)
                        if JCH == 2 and jc == 1 and bm == 1 and NBM > 1:
                            for jt in range(NJT // 2):
                                transpose_relu(jt)
                                transposed.add(jt)
                    if jc == min(0, JCH - 1) or (JCH == 1 and jc == 0):
                        # all local zc written -> tiny AllReduce, placed on the
                        # gpsimd queue between the two chunks' RS pieces
                        nc.gpsimd.collective_compute(
                            "AllReduce", mybir.AluOpType.add,
                            replica_groups=[list(range(n_cores))],
                            ins=[zc_in.opt()], outs=[zc_out.opt()])
                        # bias = bf1 + sum of the 4 zc rows
                        for g in range(4):
                            nc.gpsimd.dma_start(
                                zcb4[:, g, :],
                                zc_out[g:g + 1, :].rearrange(
                                    "o (jt p) -> (o p) jt", p=128))
                        nc.gpsimd.tensor_add(biasall[:], bf1_sb[:],
                                             zcb4[:, 0, :])
                        for g in range(1, 4):
                            nc.gpsimd.tensor_add(biasall[:], biasall[:],
                                                 zcb4[:, g, :])

                # remaining transposes, in RS-piece arrival order (blt-outer)
                for blt in range(len(zs_sb)):
                    for jt in range(NJT):
                        if jt not in transposed:
                            transpose_relu_blk(jt, blt)
                # final GEMM: out = zrT.T @ Wf2 + bf2
                for ct in range(NCT):
                    ps2 = finp.tile([128, BL], F32, name=f"fo_{ct}", tag="fin")
                    for jt in range(NJT):
                        nc.tensor.matmul(ps2[:],
                                         wf2_sb[:, jt, ct * 128:(ct + 1) * 128],
                                         zrT[jt][:], start=(jt == 0),
                                         stop=(jt == NJT - 1))
                    osb = fop.tile([128, BL], F32, name=f"osb_{ct}", tag="osb")
                    nc.vector.tensor_scalar_add(osb[:], ps2[:],
                                                bf2_sb[:, ct:ct + 1])
                    nc.scalar.dma_start(outT[ct * 128:(ct + 1) * 128, :], osb[:])

    return nc


def prep_in_maps(cfg, x, W, b, gamma, beta, Wf1, bf1, Wf2, bf2):
    d = _dims(cfg)
    n_cores, DI, DO, DF = cfg["n_cores"], cfg["DI"], cfg["DO"], cfg["DF"]
    KB, T, TPB, NJT, NCT = d["KB"], d["T"], d["TPB"], d["NJT"], d["NCT"]

    xTb = np.ascontiguousarray(np.asarray(x, dtype=np.float32).T.astype(BD))
    wf2b = np.ascontiguousarray(np.asarray(Wf2, dtype=np.float32).astype(BD))
    bf1_rr = np.ascontiguousarray(
        np.asarray(bf1, dtype=np.float32).reshape(NJT, 128).T)
    bf2_rr = np.ascontiguousarray(
        np.asarray(bf2, dtype=np.float32).reshape(NCT, 128).T)

    def fold_cols(a_loc):  # [KB, DO] -> [128, T] with col = kb*TPB+ot
        return np.ascontiguousarray(
            np.asarray(a_loc, dtype=np.float32)
            .reshape(KB, TPB, 128).transpose(2, 0, 1).reshape(128, T))

    in_maps = []
    for c in range(n_cores):
        ks = slice(c * KB, (c + 1) * KB)
        wl = np.ascontiguousarray(
            np.asarray(W[ks], dtype=np.float32).reshape(KB * DI, DO).astype(BD))
        w1l = np.ascontiguousarray(
            np.asarray(Wf1[c * KB * DO:(c + 1) * KB * DO], dtype=np.float32)
            .astype(BD))
        in_maps.append({
            "xT": xTb, "wloc": wl, "w1loc": w1l, "wf2": wf2b,
            "b_r": fold_cols(b[ks]), "gamma_r": fold_cols(gamma[ks]),
            "beta_r": fold_cols(beta[ks]),
            "bf1_r": bf1_rr, "bf2_r": bf2_rr,
        })
    return in_maps


def assemble_output(cfg, results):
    d = _dims(cfg)
    B, DO2, BL, PL = cfg["B"], cfg["DO2"], d["BL"], d["PL"]
    NBM, BMW, NPS, PR = d["NBM"], d["BMW"], d["NPS"], d["PR"]
    out = np.empty((B, DO2), dtype=np.float32)
    for c in range(cfg["n_cores"]):
        oc = results[c]["outT"].T  # [BL, DO2], local row l = (bm*NPS+sp)*PL + r
        for bm in range(NBM):
            for sp_i in range(NPS):
                l0 = (bm * NPS + sp_i) * PL
                g0 = bm * BMW + sp_i * PR + c * PL
                out[g0:g0 + PL, :] = oc[l0:l0 + PL, :]
    return out


_COMPILED = None


def _get_compiled():
    global _COMPILED
    if _COMPILED is None:
        nc = build_bass(FULL_CFG)
        nc.compile()
        _COMPILED = nc
    return _COMPILED


def kernel(**inputs):
    cfg = FULL_CFG
    nc = _get_compiled()
    in_maps = prep_in_maps(cfg, **inputs)
    res = run_bass_kernel_spmd(nc, in_maps,
                               core_ids=list(range(cfg["n_cores"])))
    return assemble_output(cfg, res.results)


# revision 32
# speedup vs baseline: 1.0596x; 1.0596x over previous
"""Trainium2 Bass kernel for nn_ExtremeOptimizationLayer (64-branch MLP + per-branch
BatchNorm + fusion gate), SPMD across 8 NeuronCores.

Sharding: expert-parallel over the 64 branches (8 per core). Per core:
  GEMM1: h_k = relu(x @ W_k + b_k) for the 8 local branches, full batch,
         computing BN batch stats on the fly (bn_stats/bn_aggr).
  BN is folded into the fusion GEMM: h_norm = h*u + v with
         u = gamma*rsqrt(var+eps), v = beta - mean*u, so
         z_partial = h @ (u*Wf1_local) + (v @ Wf1_local).
  The rank-1 term zc = v @ Wf1_local is computed with column-packed M=1
  matmuls (4 concurrent PE column groups), summed across cores by a tiny
  AllReduce, and folded into the post-RS relu bias.
  GEMM2: z_partial[b, j] over the local 8192 rows of Wf1.
  ReduceScatter (16 x 256-row pieces, overlapped with compute) sums the 8
  partial z's; each core ends with 256 batch rows, applies relu(z+bf1+zc)
  and the final GEMM with Wf2.
All matmuls run in bf16 (fp32 PSUM accumulation).

DMA queue split (avoids head-of-line blocking): weights/consts on gpsimd,
xT + h traffic on sync, z/zs/out traffic on scalar.
"""

import numpy as np
import ml_dtypes

import concourse.bass as bass
import concourse.mybir as mybir
import concourse.tile as tile
from concourse import bacc
from concourse.bass_utils import run_bass_kernel_spmd
from concourse.masks import make_identity

F32 = mybir.dt.float32
BF16 = mybir.dt.bfloat16
BD = ml_dtypes.bfloat16
AF = mybir.ActivationFunctionType

FULL_CFG = dict(
    n_cores=8, B=2048, DI=1024, DO=1024, KT=64, DF=1024, DO2=1024,
    JCH=2, EPS=1e-5,
)


def _dims(cfg):
    d = dict(cfg)
    d["KB"] = cfg["KT"] // cfg["n_cores"]        # branches per core
    d["TPB"] = cfg["DO"] // 128                  # o-tiles per branch
    d["T"] = d["KB"] * d["TPB"]                  # local ko tiles
    d["NIT"] = cfg["DI"] // 128                  # i-tiles (GEMM1 contraction)
    d["BC"] = min(512, cfg["B"])                 # GEMM1 batch chunk
    d["NBC"] = cfg["B"] // d["BC"]
    d["JW"] = cfg["DF"] // cfg["JCH"]            # j-chunk width
    d["BMW"] = min(512, cfg["B"])                # GEMM2 b-macro width
    d["NBM"] = cfg["B"] // d["BMW"]
    d["NBT"] = d["BMW"] // 128                   # b-tiles per macro
    d["PR"] = min(256, d["BMW"])                 # RS piece rows
    d["NPS"] = d["BMW"] // d["PR"]               # RS pieces per b-macro
    d["BL"] = cfg["B"] // cfg["n_cores"]         # local batch rows after RS
    d["PL"] = d["PR"] // cfg["n_cores"]          # local rows per RS piece
    d["NJT"] = cfg["DF"] // 128                  # final j-tiles
    d["NCT"] = cfg["DO2"] // 128                 # final out-col tiles
    d["NBLT"] = (d["BL"] + 127) // 128           # final local-b tiles
    d["HQ"] = min(4, d["T"])                     # ko-tiles per h2 macro-DMA
    return d


def build_bass(cfg):
    d = _dims(cfg)
    n_cores, B, DI, DO = cfg["n_cores"], cfg["B"], cfg["DI"], cfg["DO"]
    DF, DO2, JCH, EPS = cfg["DF"], cfg["DO2"], cfg["JCH"], cfg["EPS"]
    KB, TPB, T, NIT = d["KB"], d["TPB"], d["T"], d["NIT"]
    BC, NBC, JW = d["BC"], d["NBC"], d["JW"]
    BMW, NBM, NBT, BL, PL = d["BMW"], d["NBM"], d["NBT"], d["BL"], d["PL"]
    PR, NPS, HQ = d["PR"], d["NPS"], d["HQ"]
    NJT, NCT, NBLT = d["NJT"], d["NCT"], d["NBLT"]

    nc = bacc.Bacc("TRN2", target_bir_lowering=False, debug=False,
                   num_devices=n_cores)

    xT = nc.dram_tensor("xT", [DI, B], BF16, kind="ExternalInput").ap()
    wloc = nc.dram_tensor("wloc", [KB * DI, DO], BF16, kind="ExternalInput").ap()
    w1loc = nc.dram_tensor("w1loc", [KB * DO, DF], BF16, kind="ExternalInput").ap()
    wf2 = nc.dram_tensor("wf2", [DF, DO2], BF16, kind="ExternalInput").ap()
    b_r = nc.dram_tensor("b_r", [128, T], F32, kind="ExternalInput").ap()
    gamma_r = nc.dram_tensor("gamma_r", [128, T], F32, kind="ExternalInput").ap()
    beta_r = nc.dram_tensor("beta_r", [128, T], F32, kind="ExternalInput").ap()
    bf1_r = nc.dram_tensor("bf1_r", [128, NJT], F32, kind="ExternalInput").ap()
    bf2_r = nc.dram_tensor("bf2_r", [128, NCT], F32, kind="ExternalInput").ap()
    outT = nc.dram_tensor("outT", [DO2, BL], F32, kind="ExternalOutput").ap()

    h_d = nc.dram_tensor("h_d", [KB * DO, B], BF16, kind="Internal").ap()
    # RS pieces: (j-chunk, b-macro, sub-piece) -> PR rows of z
    zp = {}
    zs = {}
    for jc in range(JCH):
        for bm in range(NBM):
            for sp in range(NPS):
                zp[(jc, bm, sp)] = nc.dram_tensor(
                    f"zp{jc}_{bm}_{sp}", [PR, JW], F32, kind="Internal").ap()
                zs[(jc, bm, sp)] = nc.dram_tensor(
                    f"zs{jc}_{bm}_{sp}", [PL, JW], F32, kind="Internal").ap()
    zc_in = nc.dram_tensor("zc_in", [4, DF], F32, kind="Internal").ap()
    zc_out = nc.dram_tensor("zc_out", [4, DF], F32, kind="Internal").ap()

    with tile.TileContext(nc) as tc:
        with tc.tile_pool(name="const", bufs=1) as cp, \
             tc.tile_pool(name="stats", bufs=1) as sp_pool, \
             tc.tile_pool(name="w1a", bufs=T) as w1a, \
             tc.tile_pool(name="h2", bufs=4) as h2p:
            br_sb = cp.tile([128, T], F32, name="br_sb")
            gr_sb = cp.tile([128, T], F32, name="gr_sb")
            be_sb = cp.tile([128, T], F32, name="be_sb")
            bf1_sb = cp.tile([128, NJT], F32, name="bf1_sb")
            bf2_sb = cp.tile([128, NCT], F32, name="bf2_sb")
            eps_sb = cp.tile([128, 1], F32, name="eps_sb")
            ident = cp.tile([128, 128], F32, name="ident")
            wf2_sb = cp.tile([128, NJT, DO2], BF16, name="wf2_sb")

            # stats
            mv = sp_pool.tile([128, T, 2], F32, name="mv")
            u_all = sp_pool.tile([128, T], F32, name="u_all")
            v_f32 = sp_pool.tile([128, T], F32, name="v_f32")
            v_bf = sp_pool.tile([128, T], BF16, name="v_bf")
            zcs_t = [sp_pool.tile([128, JW], F32, name=f"zcs_{jc}")
                     for jc in range(JCH)]

            w1_tiles = [[None] * T for _ in range(JCH)]

            def zc_pack_mm(zc_ps, jc, t):
                """zc[j] += v[t-tile] @ Wf1[t-tile, jc-chunk] as an M=1 matmul
                packed into PE column group t%4 (concurrent execution)."""
                g = t % 4
                nc.tensor.matmul(zc_ps[32 * g:32 * g + 1, :], v_bf[:, t:t + 1],
                                 w1_tiles[jc][t][:],
                                 start=(t < 4), stop=(t >= T - 4),
                                 tile_position=(0, 32 * g),
                                 skip_group_check=True)

            def zc_collect(zc_ps, zcs, jc):
                """Copy the zc PSUM bank to SBUF (lane-aligned), then DMA the 4
                packed rows (partitions 0/32/64/96) to the AllReduce input."""
                nc.scalar.activation(zcs[:], zc_ps[:], AF.Copy)
                rows = zcs[:].rearrange("(g s) w -> g s w", s=32)[:, 0:1, :]
                nc.scalar.dma_start(zc_in[0:4, jc * JW:(jc + 1) * JW], rows)

            # ---------------- GEMM1: branch MLPs + BN stats ----------------
            with tc.tile_pool(name="xt", bufs=1) as xtp, \
                 tc.tile_pool(name="w", bufs=2 * NIT) as wp, \
                 tc.tile_pool(name="h1", bufs=8) as hp, \
                 tc.tile_pool(name="bn", bufs=2 * TPB + 2) as bnp, \
                 tc.tile_pool(name="g1ps", bufs=6, space="PSUM") as g1ps, \
                 tc.tile_pool(name="zc0ps", bufs=1, space="PSUM") as zc0ps:
                # branch-0 weights split across two queues for fast start
                w_tiles = []
                for it in range(NIT):
                    wt = wp.tile([128, DO], BF16, name=f"w_0_{it}", tag="w")
                    eng = nc.sync if it == 0 else (
                        nc.gpsimd if it % 2 == 0 else nc.scalar)
                    eng.dma_start(wt[:], wloc[it * 128:(it + 1) * 128, :])
                    w_tiles.append(wt)
                nc.gpsimd.dma_start(br_sb[:], b_r[:, :])
                nc.gpsimd.dma_start(gr_sb[:], gamma_r[:, :])
                nc.gpsimd.dma_start(be_sb[:], beta_r[:, :])
                nc.gpsimd.memset(eps_sb[:], EPS)
                # xT on the sync queue, bc-major so the first MMs unblock fast
                xt_sb = xtp.tile([128, NIT, B], BF16, name="xt_sb")
                for bc in range(NBC):
                    for it in range(NIT):
                        nc.sync.dma_start(
                            xt_sb[:, it, bc * BC:(bc + 1) * BC],
                            xT[it * 128:(it + 1) * 128, bc * BC:(bc + 1) * BC])
                zc0_ps = zc0ps.tile([128, JW], F32, name="zc0_ps")
                nc.vector.memset(zc0_ps[:], 0.0)

                for kb in range(KB):
                    if kb > 0:
                        w_tiles = []
                        for it in range(NIT):
                            wt = wp.tile([128, DO], BF16, name=f"w_{kb}_{it}",
                                         tag="w")
                            nc.gpsimd.dma_start(
                                wt[:],
                                wloc[kb * DI + it * 128:kb * DI + (it + 1) * 128, :])
                            w_tiles.append(wt)
                    # spread Wf1 chunk-0 prefetch across branches (gpsimd)
                    for ot in range(TPB):
                        t = kb * TPB + ot
                        w1t = w1a.tile([128, JW], BF16, name=f"w1_0_{t}",
                                       tag="w1a")
                        nc.gpsimd.dma_start(
                            w1t[:], w1loc[t * 128:(t + 1) * 128, 0:JW])
                        w1_tiles[0][t] = w1t
                    bn6s = [bnp.tile([128, NBC, 6], F32,
                                     name=f"bn6_{kb * TPB + ot}", tag="bn6")
                            for ot in range(TPB)]
                    # branch 0 runs bc-outer so the first matmuls only need the
                    # first xT batch-chunk; later branches run ot-outer
                    if kb == 0:
                        loop = [(ot, bc) for bc in range(NBC) for ot in range(TPB)]
                    else:
                        loop = [(ot, bc) for ot in range(TPB) for bc in range(NBC)]
                    for ot, bc in loop:
                        t = kb * TPB + ot
                        ps = g1ps.tile([128, BC], F32, name=f"g1_{t}_{bc}",
                                       tag="g1")
                        for it in range(NIT):
                            nc.tensor.matmul(
                                ps[:],
                                w_tiles[it][:, ot * 128:(ot + 1) * 128],
                                xt_sb[:, it, bc * BC:(bc + 1) * BC],
                                start=(it == 0), stop=(it == NIT - 1))
                        hsb = hp.tile([128, BC], BF16, name=f"h_{t}_{bc}",
                                      tag="h1")
                        nc.scalar.activation(hsb[:], ps[:], AF.Relu,
                                             bias=br_sb[:, t:t + 1])
                        nc.vector.bn_stats(bn6s[ot][:, bc, :], hsb[:])
                        nc.scalar.dma_start(
                            h_d[t * 128:(t + 1) * 128, bc * BC:(bc + 1) * BC],
                            hsb[:])
                    for ot in range(TPB):
                        t = kb * TPB + ot
                        nc.vector.bn_aggr(
                            mv[:, t, :],
                            bn6s[ot][:].rearrange("p a (x c) -> p (a x) c", c=3))
                    # per-branch BN affine folding: u = gamma*rsqrt(var+eps),
                    # v = beta - mean*u
                    t0 = kb * TPB
                    stdt = bnp.tile([128, TPB], F32, name=f"std_{kb}", tag="std")
                    nc.scalar.activation(stdt[:], mv[:, t0:t0 + TPB, 1:2],
                                         AF.Sqrt, bias=eps_sb[:])
                    invt = bnp.tile([128, TPB], F32, name=f"inv_{kb}", tag="inv")
                    nc.vector.reciprocal(invt[:], stdt[:])
                    nc.vector.tensor_mul(u_all[:, t0:t0 + TPB], invt[:],
                                         gr_sb[:, t0:t0 + TPB])
                    mut = bnp.tile([128, TPB], F32, name=f"mu_{kb}", tag="mu")
                    nc.vector.tensor_mul(mut[:], mv[:, t0:t0 + TPB, 0:1],
                                         u_all[:, t0:t0 + TPB])
                    nc.vector.tensor_sub(v_f32[:, t0:t0 + TPB],
                                         be_sb[:, t0:t0 + TPB], mut[:])
                    nc.vector.tensor_copy(v_bf[:, t0:t0 + TPB],
                                          v_f32[:, t0:t0 + TPB])
                    # chunk-0 prep (zc, then fold u into Wf1) for the PREVIOUS
                    # branch — its stats chain has finished by now, so the
                    # in-order PE doesn't stall on it.
                    for pb in ([kb - 1] if kb > 0 else []) + \
                              ([kb] if kb == KB - 1 else []):
                        for ot in range(TPB):
                            t = pb * TPB + ot
                            zc_pack_mm(zc0_ps, 0, t)
                            nc.vector.tensor_scalar_mul(w1_tiles[0][t][:],
                                                        w1_tiles[0][t][:],
                                                        u_all[:, t:t + 1])
                zc_collect(zc0_ps, zcs_t[0], 0)

            # ---------------- GEMM2: fusion gate partials + RS ----------------
            with tc.tile_pool(name="w1b", bufs=(JCH - 1) * T if JCH > 1 else 1) as w1b, \
                 tc.tile_pool(name="zsb", bufs=3) as zsbp, \
                 tc.tile_pool(name="zr", bufs=1) as zrp, \
                 tc.tile_pool(name="fo", bufs=3) as fop, \
                 tc.tile_pool(name="zps", bufs=6, space="PSUM") as zps, \
                 tc.tile_pool(name="fin_ps", bufs=2, space="PSUM") as finp:
                # consts needed from the middle of GEMM2 on (gpsimd queue)
                nc.gpsimd.dma_start(bf1_sb[:], bf1_r[:, :])
                nc.gpsimd.dma_start(bf2_sb[:], bf2_r[:, :])
                make_identity(nc, ident[:])
                # later chunks' Wf1 DMA (gpsimd queue, ahead of any RS)
                for jc in range(1, JCH):
                    for t in range(T):
                        w1t = w1b.tile([128, JW], BF16, name=f"w1_{jc}_{t}",
                                       tag="w1b")
                        nc.gpsimd.dma_start(
                            w1t[:],
                            w1loc[t * 128:(t + 1) * 128, jc * JW:(jc + 1) * JW])
                        w1_tiles[jc][t] = w1t
                for jt in range(NJT):
                    nc.gpsimd.dma_start(wf2_sb[:, jt, :],
                                        wf2[jt * 128:(jt + 1) * 128, :])

                # final-phase state (filled in as RS pieces land)
                zs_sb = []
                for blt in range(NBLT):
                    blw = min(128, BL - blt * 128)
                    zt = zrp.tile([128, DF], F32, name=f"zs_sb_{blt}",
                                  tag=f"zs_sb{blt}")
                    zs_sb.append((zt, blw))
                zcb4 = zrp.tile([128, 4, NJT], F32, name="zcb4")
                biasall = zrp.tile([128, NJT], F32, name="biasall")
                zrT = [zrp.tile([128, BL], BF16, name=f"zrT_{jt}",
                                tag=f"zrT{jt}") for jt in range(NJT)]
                transposed = set()
                pending_zs = []

                def flush_zs_loads():
                    for (jc_, bm_, sp_) in pending_zs:
                        l0 = (bm_ * NPS + sp_) * PL
                        blt, ro2 = l0 // 128, l0 % 128
                        nc.scalar.dma_start(
                            zs_sb[blt][0][ro2:ro2 + PL,
                                          jc_ * JW:(jc_ + 1) * JW],
                            zs[(jc_, bm_, sp_)][:, :])
                    pending_zs.clear()

                def transpose_relu_blk(jt, blt):
                    zt, blw = zs_sb[blt]
                    tp = finp.tile([128, 128], F32, name=f"tp_{jt}_{blt}",
                                   tag="fin")
                    nc.tensor.transpose(tp[:, 0:blw],
                                        zt[0:blw, jt * 128:(jt + 1) * 128],
                                        ident[0:blw, 0:blw])
                    nc.scalar.activation(zrT[jt][:, blt * 128:blt * 128 + blw],
                                         tp[:, 0:blw], AF.Relu,
                                         bias=biasall[:, jt:jt + 1])

                def transpose_relu(jt):
                    for blt in range(len(zs_sb)):
                        transpose_relu_blk(jt, blt)

                for jc in range(JCH):
                    for bm in range(NBM):
                        z_ps = [zps.tile([128, JW], F32, name=f"z_{jc}_{bm}_{bt}",
                                         tag="z")
                                for bt in range(NBT)]
                        for tq in range(T // HQ):
                            ht = h2p.tile([128, HQ, BMW], BF16,
                                          name=f"h2_{jc}_{bm}_{tq}", tag="h2")
                            nc.sync.dma_start(
                                ht[:],
                                h_d[tq * HQ * 128:(tq + 1) * HQ * 128,
                                    bm * BMW:(bm + 1) * BMW]
                                .rearrange("(q p) b -> p q b", p=128))
                            for q in range(HQ):
                                t = tq * HQ + q
                                for bt in range(NBT):
                                    nc.tensor.matmul(
                                        z_ps[bt][:],
                                        ht[:, q, bt * 128:(bt + 1) * 128],
                                        w1_tiles[jc][t][:],
                                        start=(t == 0), stop=(t == T - 1),
                                        skip_group_check=True)
                        for bt in range(NBT):
                            zsb = zsbp.tile([128, JW], F32,
                                            name=f"zsb_{jc}_{bm}_{bt}", tag="zsb")
                            nc.vector.tensor_copy(zsb[:], z_ps[bt][:])
                            sp_i = (bt * 128) // PR
                            ro = (bt * 128) % PR
                            nc.scalar.dma_start(
                                zp[(jc, bm, sp_i)][ro:ro + 128, :], zsb[:])
                            if ro + 128 == PR:
                                nc.gpsimd.collective_compute(
                                    "ReduceScatter", mybir.AluOpType.add,
                                    replica_groups=[list(range(n_cores))],
                                    ins=[zp[(jc, bm, sp_i)].opt()],
                                    outs=[zs[(jc, bm, sp_i)].opt()])
                                pending_zs.append((jc, bm, sp_i))
                        # prep of next chunk's zc + scale folding, right
                        # after this chunk's first b-macro (tiles have arrived,
                        # and the folded tiles are ready well before chunk nj)
                        nj = jc + 1
                        if bm == 0 and nj < JCH:
                            zcn_ps = finp.tile([128, JW], F32, name=f"zc_{nj}",
                                               tag="fin")
                            nc.vector.memset(zcn_ps[:], 0.0)
                            for t in range(T):
                                zc_pack_mm(zcn_ps, nj, t)
                            for t in range(T):
                                nc.vector.tensor_scalar_mul(
                                    w1_tiles[nj][t][:], w1_tiles[nj][t][:],
                                    u_all[:, t:t + 1])
                            zc_collect(zcn_ps, zcs_t[nj], nj)
                        # interleave the first j-half's transposes midway
                        # through the second chunk (their RS long finished)
                        if JCH == 2 and jc == 1 and bm == 1 and NBM > 1:
                            for jt in range(NJT // 2):
                                transpose_relu(jt)
                                transposed.add(jt)
                    if jc == min(0, JCH - 1) or (JCH == 1 and jc == 0):
                        # all local zc written -> tiny AllReduce, placed on the
                        # gpsimd queue between the two chunks' RS pieces
                        nc.gpsimd.collective_compute(
                            "AllReduce", mybir.AluOpType.add,
                            replica_groups=[list(range(n_cores))],
                            ins=[zc_in.opt()], outs=[zc_out.opt()])
                        # bias = bf1 + sum of the 4 zc rows
                        flush_zs_loads()
                        for g in range(4):
                            nc.scalar.dma_start(
                                zcb4[:, g, :],
                                zc_out[g:g + 1, :].rearrange(
                                    "o (jt p) -> (o p) jt", p=128))
                        nc.gpsimd.tensor_add(biasall[:], bf1_sb[:],
                                             zcb4[:, 0, :])
                        for g in range(1, 4):
                            nc.gpsimd.tensor_add(biasall[:], biasall[:],
                                                 zcb4[:, g, :])

                flush_zs_loads()
                # remaining transposes, in RS-piece arrival order (blt-outer)
                for blt in range(len(zs_sb)):
                    for jt in range(NJT):
                        if jt not in transposed:
                            transpose_relu_blk(jt, blt)
                # final GEMM: out = zrT.T @ Wf2 + bf2
                for ct in range(NCT):
                    ps2 = finp.tile([128, BL], F32, name=f"fo_{ct}", tag="fin")
                    for jt in range(NJT):
                        nc.tensor.matmul(ps2[:],
                                         wf2_sb[:, jt, ct * 128:(ct + 1) * 128],
                                         zrT[jt][:], start=(jt == 0),
                                         stop=(jt == NJT - 1))
                    osb = fop.tile([128, BL], F32, name=f"osb_{ct}", tag="osb")
                    nc.vector.tensor_scalar_add(osb[:], ps2[:],
                                                bf2_sb[:, ct:ct + 1])
                    nc.scalar.dma_start(outT[ct * 128:(ct + 1) * 128, :], osb[:])

    return nc


def prep_in_maps(cfg, x, W, b, gamma, beta, Wf1, bf1, Wf2, bf2):
    d = _dims(cfg)
    n_cores, DI, DO, DF = cfg["n_cores"], cfg["DI"], cfg["DO"], cfg["DF"]
    KB, T, TPB, NJT, NCT = d["KB"], d["T"], d["TPB"], d["NJT"], d["NCT"]

    xTb = np.ascontiguousarray(np.asarray(x, dtype=np.float32).T.astype(BD))
    wf2b = np.ascontiguousarray(np.asarray(Wf2, dtype=np.float32).astype(BD))
    bf1_rr = np.ascontiguousarray(
        np.asarray(bf1, dtype=np.float32).reshape(NJT, 128).T)
    bf2_rr = np.ascontiguousarray(
        np.asarray(bf2, dtype=np.float32).reshape(NCT, 128).T)

    def fold_cols(a_loc):  # [KB, DO] -> [128, T] with col = kb*TPB+ot
        return np.ascontiguousarray(
            np.asarray(a_loc, dtype=np.float32)
            .reshape(KB, TPB, 128).transpose(2, 0, 1).reshape(128, T))

    in_maps = []
    for c in range(n_cores):
        ks = slice(c * KB, (c + 1) * KB)
        wl = np.ascontiguousarray(
            np.asarray(W[ks], dtype=np.float32).reshape(KB * DI, DO).astype(BD))
        w1l = np.ascontiguousarray(
            np.asarray(Wf1[c * KB * DO:(c + 1) * KB * DO], dtype=np.float32)
            .astype(BD))
        in_maps.append({
            "xT": xTb, "wloc": wl, "w1loc": w1l, "wf2": wf2b,
            "b_r": fold_cols(b[ks]), "gamma_r": fold_cols(gamma[ks]),
            "beta_r": fold_cols(beta[ks]),
            "bf1_r": bf1_rr, "bf2_r": bf2_rr,
        })
    return in_maps


def assemble_output(cfg, results):
    d = _dims(cfg)
    B, DO2, BL, PL = cfg["B"], cfg["DO2"], d["BL"], d["PL"]
    NBM, BMW, NPS, PR = d["NBM"], d["BMW"], d["NPS"], d["PR"]
    out = np.empty((B, DO2), dtype=np.float32)
    for c in range(cfg["n_cores"]):
        oc = results[c]["outT"].T  # [BL, DO2], local row l = (bm*NPS+sp)*PL + r
        for bm in range(NBM):
            for sp_i in range(NPS):
                l0 = (bm * NPS + sp_i) * PL
                g0 = bm * BMW + sp_i * PR + c * PL
                out[g0:g0 + PL, :] = oc[l0:l0 + PL, :]
    return out


_COMPILED = None


def _get_compiled():
    global _COMPILED
    if _COMPILED is None:
        nc = build_bass(FULL_CFG)
        nc.compile()
        _COMPILED = nc
    return _COMPILED


def kernel(**inputs):
    cfg = FULL_CFG
    nc = _get_compiled()
    in_maps = prep_in_maps(cfg, **inputs)
    res = run_bass_kernel_spmd(nc, in_maps,
                               core_ids=list(range(cfg["n_cores"])))
    return assemble_output(cfg, res.results)


# revision 33
# speedup vs baseline: 1.0618x; 1.0021x over previous
"""Trainium2 Bass kernel for nn_ExtremeOptimizationLayer (64-branch MLP + per-branch
BatchNorm + fusion gate), SPMD across 8 NeuronCores.

Sharding: expert-parallel over the 64 branches (8 per core). Per core:
  GEMM1: h_k = relu(x @ W_k + b_k) for the 8 local branches, full batch,
         computing BN batch stats on the fly (bn_stats/bn_aggr).
  BN is folded into the fusion GEMM: h_norm = h*u + v with
         u = gamma*rsqrt(var+eps), v = beta - mean*u, so
         z_partial = h @ (u*Wf1_local) + (v @ Wf1_local).
  The rank-1 term zc = v @ Wf1_local is computed with column-packed M=1
  matmuls (4 concurrent PE column groups), summed across cores by a tiny
  AllReduce, and folded into the post-RS relu bias.
  GEMM2: z_partial[b, j] over the local 8192 rows of Wf1.
  ReduceScatter (16 x 256-row pieces, overlapped with compute) sums the 8
  partial z's; each core ends with 256 batch rows, applies relu(z+bf1+zc)
  and the final GEMM with Wf2.
All matmuls run in bf16 (fp32 PSUM accumulation).

DMA queue split (avoids head-of-line blocking): weights/consts on gpsimd,
xT + h traffic on sync, z/zs/out traffic on scalar.
"""

import numpy as np
import ml_dtypes

import concourse.bass as bass
import concourse.mybir as mybir
import concourse.tile as tile
from concourse import bacc
from concourse.bass_utils import run_bass_kernel_spmd
from concourse.masks import make_identity

F32 = mybir.dt.float32
BF16 = mybir.dt.bfloat16
BD = ml_dtypes.bfloat16
AF = mybir.ActivationFunctionType

FULL_CFG = dict(
    n_cores=8, B=2048, DI=1024, DO=1024, KT=64, DF=1024, DO2=1024,
    JCH=2, EPS=1e-5,
)


def _dims(cfg):
    d = dict(cfg)
    d["KB"] = cfg["KT"] // cfg["n_cores"]        # branches per core
    d["TPB"] = cfg["DO"] // 128                  # o-tiles per branch
    d["T"] = d["KB"] * d["TPB"]                  # local ko tiles
    d["NIT"] = cfg["DI"] // 128                  # i-tiles (GEMM1 contraction)
    d["BC"] = min(512, cfg["B"])                 # GEMM1 batch chunk
    d["NBC"] = cfg["B"] // d["BC"]
    d["JW"] = cfg["DF"] // cfg["JCH"]            # j-chunk width
    d["BMW"] = min(512, cfg["B"])                # GEMM2 b-macro width
    d["NBM"] = cfg["B"] // d["BMW"]
    d["NBT"] = d["BMW"] // 128                   # b-tiles per macro
    d["PR"] = min(256, d["BMW"])                 # RS piece rows
    d["NPS"] = d["BMW"] // d["PR"]               # RS pieces per b-macro
    d["BL"] = cfg["B"] // cfg["n_cores"]         # local batch rows after RS
    d["PL"] = d["PR"] // cfg["n_cores"]          # local rows per RS piece
    d["NJT"] = cfg["DF"] // 128                  # final j-tiles
    d["NCT"] = cfg["DO2"] // 128                 # final out-col tiles
    d["NBLT"] = (d["BL"] + 127) // 128           # final local-b tiles
    d["HQ"] = min(4, d["T"])                     # ko-tiles per h2 macro-DMA
    return d


def build_bass(cfg):
    d = _dims(cfg)
    n_cores, B, DI, DO = cfg["n_cores"], cfg["B"], cfg["DI"], cfg["DO"]
    DF, DO2, JCH, EPS = cfg["DF"], cfg["DO2"], cfg["JCH"], cfg["EPS"]
    KB, TPB, T, NIT = d["KB"], d["TPB"], d["T"], d["NIT"]
    BC, NBC, JW = d["BC"], d["NBC"], d["JW"]
    BMW, NBM, NBT, BL, PL = d["BMW"], d["NBM"], d["NBT"], d["BL"], d["PL"]
    PR, NPS, HQ = d["PR"], d["NPS"], d["HQ"]
    NJT, NCT, NBLT = d["NJT"], d["NCT"], d["NBLT"]

    nc = bacc.Bacc("TRN2", target_bir_lowering=False, debug=False,
                   num_devices=n_cores)

    xT = nc.dram_tensor("xT", [DI, B], BF16, kind="ExternalInput").ap()
    wloc = nc.dram_tensor("wloc", [KB * DI, DO], BF16, kind="ExternalInput").ap()
    w1loc = nc.dram_tensor("w1loc", [KB * DO, DF], BF16, kind="ExternalInput").ap()
    wf2 = nc.dram_tensor("wf2", [DF, DO2], BF16, kind="ExternalInput").ap()
    b_r = nc.dram_tensor("b_r", [128, T], F32, kind="ExternalInput").ap()
    gamma_r = nc.dram_tensor("gamma_r", [128, T], F32, kind="ExternalInput").ap()
    beta_r = nc.dram_tensor("beta_r", [128, T], F32, kind="ExternalInput").ap()
    bf1_r = nc.dram_tensor("bf1_r", [128, NJT], F32, kind="ExternalInput").ap()
    bf2_r = nc.dram_tensor("bf2_r", [128, NCT], F32, kind="ExternalInput").ap()
    outT = nc.dram_tensor("outT", [DO2, BL], F32, kind="ExternalOutput").ap()

    h_d = nc.dram_tensor("h_d", [KB * DO, B], BF16, kind="Internal").ap()

    def pieces(bm):
        # (sub-piece index, row0 within macro, nrows); the last b-macro is one
        # full piece so its (latency-critical) RS is a single collective
        if bm == NBM - 1 or NPS == 1:
            return [(0, 0, BMW)]
        return [(sp, sp * PR, PR) for sp in range(NPS)]

    # RS pieces: (j-chunk, b-macro, sub-piece)
    zp = {}
    zs = {}
    for jc in range(JCH):
        for bm in range(NBM):
            for sp, row0, nrows in pieces(bm):
                zp[(jc, bm, sp)] = nc.dram_tensor(
                    f"zp{jc}_{bm}_{sp}", [nrows, JW], F32, kind="Internal").ap()
                zs[(jc, bm, sp)] = nc.dram_tensor(
                    f"zs{jc}_{bm}_{sp}", [nrows // n_cores, JW], F32,
                    kind="Internal").ap()
    zc_in = nc.dram_tensor("zc_in", [4, DF], F32, kind="Internal").ap()
    zc_out = nc.dram_tensor("zc_out", [4, DF], F32, kind="Internal").ap()

    with tile.TileContext(nc) as tc:
        with tc.tile_pool(name="const", bufs=1) as cp, \
             tc.tile_pool(name="stats", bufs=1) as sp_pool, \
             tc.tile_pool(name="w1a", bufs=T) as w1a, \
             tc.tile_pool(name="h2", bufs=5) as h2p:
            br_sb = cp.tile([128, T], F32, name="br_sb")
            gr_sb = cp.tile([128, T], F32, name="gr_sb")
            be_sb = cp.tile([128, T], F32, name="be_sb")
            bf1_sb = cp.tile([128, NJT], F32, name="bf1_sb")
            bf2_sb = cp.tile([128, NCT], F32, name="bf2_sb")
            eps_sb = cp.tile([128, 1], F32, name="eps_sb")
            ident = cp.tile([128, 128], F32, name="ident")
            wf2_sb = cp.tile([128, NJT, DO2], BF16, name="wf2_sb")

            # stats
            mv = sp_pool.tile([128, T, 2], F32, name="mv")
            u_all = sp_pool.tile([128, T], F32, name="u_all")
            v_f32 = sp_pool.tile([128, T], F32, name="v_f32")
            v_bf = sp_pool.tile([128, T], BF16, name="v_bf")
            zcs_t = [sp_pool.tile([128, JW], F32, name=f"zcs_{jc}")
                     for jc in range(JCH)]

            w1_tiles = [[None] * T for _ in range(JCH)]

            def zc_pack_mm(zc_ps, jc, t):
                """zc[j] += v[t-tile] @ Wf1[t-tile, jc-chunk] as an M=1 matmul
                packed into PE column group t%4 (concurrent execution)."""
                g = t % 4
                nc.tensor.matmul(zc_ps[32 * g:32 * g + 1, :], v_bf[:, t:t + 1],
                                 w1_tiles[jc][t][:],
                                 start=(t < 4), stop=(t >= T - 4),
                                 tile_position=(0, 32 * g),
                                 skip_group_check=True)

            def zc_collect(zc_ps, zcs, jc):
                """Copy the zc PSUM bank to SBUF (lane-aligned), then DMA the 4
                packed rows (partitions 0/32/64/96) to the AllReduce input."""
                nc.scalar.activation(zcs[:], zc_ps[:], AF.Copy)
                rows = zcs[:].rearrange("(g s) w -> g s w", s=32)[:, 0:1, :]
                nc.scalar.dma_start(zc_in[0:4, jc * JW:(jc + 1) * JW], rows)

            # ---------------- GEMM1: branch MLPs + BN stats ----------------
            with tc.tile_pool(name="xt", bufs=1) as xtp, \
                 tc.tile_pool(name="w", bufs=2 * NIT) as wp, \
                 tc.tile_pool(name="h1", bufs=8) as hp, \
                 tc.tile_pool(name="bn", bufs=2 * TPB + 2) as bnp, \
                 tc.tile_pool(name="g1ps", bufs=6, space="PSUM") as g1ps, \
                 tc.tile_pool(name="zc0ps", bufs=1, space="PSUM") as zc0ps:
                # branch-0 weights split across two queues for fast start
                w_tiles = []
                for it in range(NIT):
                    wt = wp.tile([128, DO], BF16, name=f"w_0_{it}", tag="w")
                    eng = nc.sync if it == 0 else (
                        nc.gpsimd if it % 2 == 0 else nc.scalar)
                    eng.dma_start(wt[:], wloc[it * 128:(it + 1) * 128, :])
                    w_tiles.append(wt)
                nc.gpsimd.dma_start(br_sb[:], b_r[:, :])
                nc.gpsimd.dma_start(gr_sb[:], gamma_r[:, :])
                nc.gpsimd.dma_start(be_sb[:], beta_r[:, :])
                nc.gpsimd.memset(eps_sb[:], EPS)
                # xT on the sync queue, bc-major so the first MMs unblock fast
                xt_sb = xtp.tile([128, NIT, B], BF16, name="xt_sb")
                for bc in range(NBC):
                    for it in range(NIT):
                        eng = nc.scalar if (bc == 0 and it % 2 == 1) else nc.sync
                        eng.dma_start(
                            xt_sb[:, it, bc * BC:(bc + 1) * BC],
                            xT[it * 128:(it + 1) * 128, bc * BC:(bc + 1) * BC])
                zc0_ps = zc0ps.tile([128, JW], F32, name="zc0_ps")
                nc.vector.memset(zc0_ps[:], 0.0)

                for kb in range(KB):
                    if kb > 0:
                        w_tiles = []
                        for it in range(NIT):
                            wt = wp.tile([128, DO], BF16, name=f"w_{kb}_{it}",
                                         tag="w")
                            nc.gpsimd.dma_start(
                                wt[:],
                                wloc[kb * DI + it * 128:kb * DI + (it + 1) * 128, :])
                            w_tiles.append(wt)
                    # spread Wf1 chunk-0 prefetch across branches (gpsimd)
                    for ot in range(TPB):
                        t = kb * TPB + ot
                        w1t = w1a.tile([128, JW], BF16, name=f"w1_0_{t}",
                                       tag="w1a")
                        nc.gpsimd.dma_start(
                            w1t[:], w1loc[t * 128:(t + 1) * 128, 0:JW])
                        w1_tiles[0][t] = w1t
                    bn6s = [bnp.tile([128, NBC, 6], F32,
                                     name=f"bn6_{kb * TPB + ot}", tag="bn6")
                            for ot in range(TPB)]
                    # branch 0 runs bc-outer so the first matmuls only need the
                    # first xT batch-chunk; later branches run ot-outer
                    if kb == 0:
                        loop = [(ot, bc) for bc in range(NBC) for ot in range(TPB)]
                    else:
                        loop = [(ot, bc) for ot in range(TPB) for bc in range(NBC)]
                    for ot, bc in loop:
                        t = kb * TPB + ot
                        ps = g1ps.tile([128, BC], F32, name=f"g1_{t}_{bc}",
                                       tag="g1")
                        for it in range(NIT):
                            nc.tensor.matmul(
                                ps[:],
                                w_tiles[it][:, ot * 128:(ot + 1) * 128],
                                xt_sb[:, it, bc * BC:(bc + 1) * BC],
                                start=(it == 0), stop=(it == NIT - 1))
                        hsb = hp.tile([128, BC], BF16, name=f"h_{t}_{bc}",
                                      tag="h1")
                        nc.scalar.activation(hsb[:], ps[:], AF.Relu,
                                             bias=br_sb[:, t:t + 1])
                        nc.vector.bn_stats(bn6s[ot][:, bc, :], hsb[:])
                        nc.scalar.dma_start(
                            h_d[t * 128:(t + 1) * 128, bc * BC:(bc + 1) * BC],
                            hsb[:])
                    for ot in range(TPB):
                        t = kb * TPB + ot
                        nc.vector.bn_aggr(
                            mv[:, t, :],
                            bn6s[ot][:].rearrange("p a (x c) -> p (a x) c", c=3))
                    # per-branch BN affine folding: u = gamma*rsqrt(var+eps),
                    # v = beta - mean*u
                    t0 = kb * TPB
                    stdt = bnp.tile([128, TPB], F32, name=f"std_{kb}", tag="std")
                    nc.scalar.activation(stdt[:], mv[:, t0:t0 + TPB, 1:2],
                                         AF.Sqrt, bias=eps_sb[:])
                    invt = bnp.tile([128, TPB], F32, name=f"inv_{kb}", tag="inv")
                    nc.vector.reciprocal(invt[:], stdt[:])
                    nc.vector.tensor_mul(u_all[:, t0:t0 + TPB], invt[:],
                                         gr_sb[:, t0:t0 + TPB])
                    mut = bnp.tile([128, TPB], F32, name=f"mu_{kb}", tag="mu")
                    nc.vector.tensor_mul(mut[:], mv[:, t0:t0 + TPB, 0:1],
                                         u_all[:, t0:t0 + TPB])
                    nc.vector.tensor_sub(v_f32[:, t0:t0 + TPB],
                                         be_sb[:, t0:t0 + TPB], mut[:])
                    nc.vector.tensor_copy(v_bf[:, t0:t0 + TPB],
                                          v_f32[:, t0:t0 + TPB])
                    # chunk-0 prep (zc, then fold u into Wf1) for the PREVIOUS
                    # branch — its stats chain has finished by now, so the
                    # in-order PE doesn't stall on it.
                    for pb in ([kb - 1] if kb > 0 else []) + \
                              ([kb] if kb == KB - 1 else []):
                        for ot in range(TPB):
                            t = pb * TPB + ot
                            zc_pack_mm(zc0_ps, 0, t)
                            nc.vector.tensor_scalar_mul(w1_tiles[0][t][:],
                                                        w1_tiles[0][t][:],
                                                        u_all[:, t:t + 1])
                zc_collect(zc0_ps, zcs_t[0], 0)

            # ---------------- GEMM2: fusion gate partials + RS ----------------
            with tc.tile_pool(name="w1b", bufs=(JCH - 1) * T if JCH > 1 else 1) as w1b, \
                 tc.tile_pool(name="zsb", bufs=3) as zsbp, \
                 tc.tile_pool(name="zr", bufs=1) as zrp, \
                 tc.tile_pool(name="fo", bufs=3) as fop, \
                 tc.tile_pool(name="zps", bufs=6, space="PSUM") as zps, \
                 tc.tile_pool(name="fin_ps", bufs=2, space="PSUM") as finp:
                # consts needed from the middle of GEMM2 on (gpsimd queue)
                nc.gpsimd.dma_start(bf1_sb[:], bf1_r[:, :])
                nc.gpsimd.dma_start(bf2_sb[:], bf2_r[:, :])
                make_identity(nc, ident[:])
                # later chunks' Wf1 DMA (gpsimd queue, ahead of any RS)
                for jc in range(1, JCH):
                    for t in range(T):
                        w1t = w1b.tile([128, JW], BF16, name=f"w1_{jc}_{t}",
                                       tag="w1b")
                        nc.gpsimd.dma_start(
                            w1t[:],
                            w1loc[t * 128:(t + 1) * 128, jc * JW:(jc + 1) * JW])
                        w1_tiles[jc][t] = w1t
                for jt in range(NJT):
                    nc.gpsimd.dma_start(wf2_sb[:, jt, :],
                                        wf2[jt * 128:(jt + 1) * 128, :])

                # final-phase state (filled in as RS pieces land)
                zs_sb = []
                for blt in range(NBLT):
                    blw = min(128, BL - blt * 128)
                    zt = zrp.tile([128, DF], F32, name=f"zs_sb_{blt}",
                                  tag=f"zs_sb{blt}")
                    zs_sb.append((zt, blw))
                zcb4 = zrp.tile([128, 4, NJT], F32, name="zcb4")
                biasall = zrp.tile([128, NJT], F32, name="biasall")
                zrT = [zrp.tile([128, BL], BF16, name=f"zrT_{jt}",
                                tag=f"zrT{jt}") for jt in range(NJT)]
                transposed = set()
                pending_zs = []

                def flush_zs_loads():
                    for (jc_, bm_, sp_) in pending_zs:
                        _, row0, nrows = next(p for p in pieces(bm_)
                                              if p[0] == sp_)
                        plp = nrows // n_cores
                        l0 = (bm_ * BMW + row0) // n_cores
                        blt, ro2 = l0 // 128, l0 % 128
                        nc.scalar.dma_start(
                            zs_sb[blt][0][ro2:ro2 + plp,
                                          jc_ * JW:(jc_ + 1) * JW],
                            zs[(jc_, bm_, sp_)][:, :])
                    pending_zs.clear()

                def transpose_relu_blk(jt, blt):
                    zt, blw = zs_sb[blt]
                    tp = finp.tile([128, 128], F32, name=f"tp_{jt}_{blt}",
                                   tag="fin")
                    nc.tensor.transpose(tp[:, 0:blw],
                                        zt[0:blw, jt * 128:(jt + 1) * 128],
                                        ident[0:blw, 0:blw])
                    nc.scalar.activation(zrT[jt][:, blt * 128:blt * 128 + blw],
                                         tp[:, 0:blw], AF.Relu,
                                         bias=biasall[:, jt:jt + 1])

                def transpose_relu(jt):
                    for blt in range(len(zs_sb)):
                        transpose_relu_blk(jt, blt)

                for jc in range(JCH):
                    for bm in range(NBM):
                        z_ps = [zps.tile([128, JW], F32, name=f"z_{jc}_{bm}_{bt}",
                                         tag="z")
                                for bt in range(NBT)]
                        for tq in range(T // HQ):
                            ht = h2p.tile([128, HQ, BMW], BF16,
                                          name=f"h2_{jc}_{bm}_{tq}", tag="h2")
                            nc.sync.dma_start(
                                ht[:],
                                h_d[tq * HQ * 128:(tq + 1) * HQ * 128,
                                    bm * BMW:(bm + 1) * BMW]
                                .rearrange("(q p) b -> p q b", p=128))
                            for q in range(HQ):
                                t = tq * HQ + q
                                for bt in range(NBT):
                                    nc.tensor.matmul(
                                        z_ps[bt][:],
                                        ht[:, q, bt * 128:(bt + 1) * 128],
                                        w1_tiles[jc][t][:],
                                        start=(t == 0), stop=(t == T - 1),
                                        skip_group_check=True)
                        for bt in range(NBT):
                            zsb = zsbp.tile([128, JW], F32,
                                            name=f"zsb_{jc}_{bm}_{bt}", tag="zsb")
                            nc.vector.tensor_copy(zsb[:], z_ps[bt][:])
                            sp_i, row0, nrows = next(
                                p for p in pieces(bm)
                                if p[1] <= bt * 128 < p[1] + p[2])
                            ro = bt * 128 - row0
                            nc.scalar.dma_start(
                                zp[(jc, bm, sp_i)][ro:ro + 128, :], zsb[:])
                            if ro + 128 == nrows:
                                nc.gpsimd.collective_compute(
                                    "ReduceScatter", mybir.AluOpType.add,
                                    replica_groups=[list(range(n_cores))],
                                    ins=[zp[(jc, bm, sp_i)].opt()],
                                    outs=[zs[(jc, bm, sp_i)].opt()])
                                pending_zs.append((jc, bm, sp_i))
                        # prep of next chunk's zc + scale folding, right
                        # after this chunk's first b-macro (tiles have arrived,
                        # and the folded tiles are ready well before chunk nj)
                        nj = jc + 1
                        if bm == 0 and nj < JCH:
                            zcn_ps = finp.tile([128, JW], F32, name=f"zc_{nj}",
                                               tag="fin")
                            nc.vector.memset(zcn_ps[:], 0.0)
                            for t in range(T):
                                zc_pack_mm(zcn_ps, nj, t)
                            for t in range(T):
                                nc.vector.tensor_scalar_mul(
                                    w1_tiles[nj][t][:], w1_tiles[nj][t][:],
                                    u_all[:, t:t + 1])
                            zc_collect(zcn_ps, zcs_t[nj], nj)
                        # interleave the first j-half's transposes midway
                        # through the second chunk (their RS long finished)
                        if JCH == 2 and jc == 1 and bm == 1 and NBM > 1:
                            for jt in range(NJT // 2):
                                transpose_relu(jt)
                                transposed.add(jt)
                    if jc == min(0, JCH - 1) or (JCH == 1 and jc == 0):
                        # all local zc written -> tiny AllReduce, placed on the
                        # gpsimd queue between the two chunks' RS pieces
                        nc.gpsimd.collective_compute(
                            "AllReduce", mybir.AluOpType.add,
                            replica_groups=[list(range(n_cores))],
                            ins=[zc_in.opt()], outs=[zc_out.opt()])
                        # bias = bf1 + sum of the 4 zc rows
                        flush_zs_loads()
                        for g in range(4):
                            nc.scalar.dma_start(
                                zcb4[:, g, :],
                                zc_out[g:g + 1, :].rearrange(
                                    "o (jt p) -> (o p) jt", p=128))
                        nc.gpsimd.tensor_add(biasall[:], bf1_sb[:],
                                             zcb4[:, 0, :])
                        for g in range(1, 4):
                            nc.gpsimd.tensor_add(biasall[:], biasall[:],
                                                 zcb4[:, g, :])

                flush_zs_loads()
                # remaining transposes, in RS-piece arrival order (blt-outer)
                for blt in range(len(zs_sb)):
                    for jt in range(NJT):
                        if jt not in transposed:
                            transpose_relu_blk(jt, blt)
                # final GEMM: out = zrT.T @ Wf2 + bf2
                for ct in range(NCT):
                    ps2 = finp.tile([128, BL], F32, name=f"fo_{ct}", tag="fin")
                    for jt in range(NJT):
                        nc.tensor.matmul(ps2[:],
                                         wf2_sb[:, jt, ct * 128:(ct + 1) * 128],
                                         zrT[jt][:], start=(jt == 0),
                                         stop=(jt == NJT - 1))
                    osb = fop.tile([128, BL], F32, name=f"osb_{ct}", tag="osb")
                    nc.vector.tensor_scalar_add(osb[:], ps2[:],
                                                bf2_sb[:, ct:ct + 1])
                    nc.scalar.dma_start(outT[ct * 128:(ct + 1) * 128, :], osb[:])

    return nc


def prep_in_maps(cfg, x, W, b, gamma, beta, Wf1, bf1, Wf2, bf2):
    d = _dims(cfg)
    n_cores, DI, DO, DF = cfg["n_cores"], cfg["DI"], cfg["DO"], cfg["DF"]
    KB, T, TPB, NJT, NCT = d["KB"], d["T"], d["TPB"], d["NJT"], d["NCT"]

    xTb = np.ascontiguousarray(np.asarray(x, dtype=np.float32).T.astype(BD))
    wf2b = np.ascontiguousarray(np.asarray(Wf2, dtype=np.float32).astype(BD))
    bf1_rr = np.ascontiguousarray(
        np.asarray(bf1, dtype=np.float32).reshape(NJT, 128).T)
    bf2_rr = np.ascontiguousarray(
        np.asarray(bf2, dtype=np.float32).reshape(NCT, 128).T)

    def fold_cols(a_loc):  # [KB, DO] -> [128, T] with col = kb*TPB+ot
        return np.ascontiguousarray(
            np.asarray(a_loc, dtype=np.float32)
            .reshape(KB, TPB, 128).transpose(2, 0, 1).reshape(128, T))

    in_maps = []
    for c in range(n_cores):
        ks = slice(c * KB, (c + 1) * KB)
        wl = np.ascontiguousarray(
            np.asarray(W[ks], dtype=np.float32).reshape(KB * DI, DO).astype(BD))
        w1l = np.ascontiguousarray(
            np.asarray(Wf1[c * KB * DO:(c + 1) * KB * DO], dtype=np.float32)
            .astype(BD))
        in_maps.append({
            "xT": xTb, "wloc": wl, "w1loc": w1l, "wf2": wf2b,
            "b_r": fold_cols(b[ks]), "gamma_r": fold_cols(gamma[ks]),
            "beta_r": fold_cols(beta[ks]),
            "bf1_r": bf1_rr, "bf2_r": bf2_rr,
        })
    return in_maps


def assemble_output(cfg, results):
    d = _dims(cfg)
    B, DO2, n_cores = cfg["B"], cfg["DO2"], cfg["n_cores"]
    NBM, BMW, NPS, PR = d["NBM"], d["BMW"], d["NPS"], d["PR"]
    out = np.empty((B, DO2), dtype=np.float32)
    for c in range(n_cores):
        oc = results[c]["outT"].T  # [BL, DO2]
        for bm in range(NBM):
            if bm == NBM - 1 or NPS == 1:
                plist = [(0, 0, BMW)]
            else:
                plist = [(sp, sp * PR, PR) for sp in range(NPS)]
            for _, row0, nrows in plist:
                plp = nrows // n_cores
                l0 = (bm * BMW + row0) // n_cores
                g0 = bm * BMW + row0 + c * plp
                out[g0:g0 + plp, :] = oc[l0:l0 + plp, :]
    return out


_COMPILED = None


def _get_compiled():
    global _COMPILED
    if _COMPILED is None:
        nc = build_bass(FULL_CFG)
        nc.compile()
        _COMPILED = nc
    return _COMPILED


def kernel(**inputs):
    cfg = FULL_CFG
    nc = _get_compiled()
    in_maps = prep_in_maps(cfg, **inputs)
    res = run_bass_kernel_spmd(nc, in_maps,
                               core_ids=list(range(cfg["n_cores"])))
    return assemble_output(cfg, res.results)
